# revision 53
# baseline (speedup 1.0000x reference)
import sys

sys.path.insert(0, "/opt/trn_rl_repo")

import numpy as np
import ml_dtypes

BF16 = ml_dtypes.bfloat16

# problem constants (hardcoded per contract)
BSZ, SEQ, E = 2, 4096, 768
NH, HD = 12, 64
NPAIR = 3      # real head pairs per core (6 heads, 2x64 dims -> 128 partitions)
NSW = 6        # sweeps: 3 pairs x 2 query-subhalves of 1024
QPC = 1024     # query rows per sweep
QCC = 2048     # query rows per core (half of seq-half... 2048 of 4096)
EH = 384       # embed slice per core (6 heads x 64)
NKC = 32       # k chunks of 128
NEC = 6        # embed chunks of 128 (contraction)
NEH = 3        # 128-chunks in EH

# Schraudolph bf16-exp constants: exp(x) ~= bitcast_int16_to_bf16(
#   round(x * 128/ln2 + (127*128 - C)));  C tuned for min max-rel-err
SCH_A = 128.0 / float(np.log(2.0))
SCH_C = 5.57
SCH_B = 127.0 * 128.0 - SCH_C

# engine-split patterns (compile-time):
#  - exp tile (j,h) -> DVE (Schraudolph) vs ACT (exact exp)
#  - denominator of tile (j,h) -> PE ones-matmul vs DVE running-sum add


def EXP_DVE(sw, j, h):
    # DVE share of exp tiles: slightly higher in emission-free sweeps
    if h != 1:
        return False
    if sw in (2, 4):
        return (j % 7) in (0, 3)
    if sw == 5:
        return (j % 4) == 0
    return (j % 5) == 0


def DEN_PE(sw, j, h):
    # a small slice of denominator accumulation rides TensorE's dependency
    # stalls; the bulk stays as DVE running-sum adds. Sweeps 0/1/3 keep the
    # d bank free during the j-loop so K/V emission chains can pair across
    # the em+d banks.
    if sw not in (2, 4):
        return False
    return ((2 * j + h) % 16) == 0


_cache = {}


def _install_drain_patch(tile, mybir):
    from concourse.vector_clock import ScopedClock

    if getattr(tile.TileContext._drain_and_barrier, "_split_waits", False):
        return

    def _drain_and_barrier(self, tick_clock, wait_clock):
        drain_inst = self.nc.sync.drain()
        wait_clock.add_sem_waits(
            drain_inst.ins, ScopedClock({None: tick_clock.global_clock})
        )
        si = drain_inst.ins.sync_info
        waits = list(si.on_wait) if si is not None else []
        if len(waits) > 1:
            # walrus TPB_CTRL codegen rejects drains with multiple sem
            # waits; split into a chain of single-wait drains
            si.on_wait = [waits[0]]
            for w in waits[1:]:
                d2 = self.nc.sync.drain()
                if d2.ins.sync_info is None:
                    d2.ins.sync_info = mybir.SyncInfo(on_wait=[w], on_update=[])
                else:
                    d2.ins.sync_info.on_wait = [w]
        self.nc.all_engine_barrier()
        assert self.sems is not None
        popped = self.nc._tile_sem_poison_stack.pop()
        assert popped is self._sem_poison
        self.nc.clear_and_free_semaphores(list(self.sems.allocated().values()))
        self.nc.all_engine_barrier()

    _drain_and_barrier._split_waits = True
    tile.TileContext._drain_and_barrier = _drain_and_barrier


def _build():
    import concourse.bass as bass
    import concourse.tile as tile
    from concourse import library_config, mybir

    _install_drain_patch(tile, mybir)

    f32 = mybir.dt.float32
    bf16 = mybir.dt.bfloat16
    i16 = mybir.dt.int16
    Exp = mybir.ActivationFunctionType.Exp
    add = mybir.AluOpType.add
    mult = mybir.AluOpType.mult

    nc = bass.Bass()
    xt = nc.declare_dram_parameter("xt", [E, SEQ], bf16, isOutput=False)
    wqt = nc.declare_dram_parameter("wqt", [E, EH], bf16, isOutput=False)
    wkt = nc.declare_dram_parameter("wkt", [E, EH], bf16, isOutput=False)
    wvt = nc.declare_dram_parameter("wvt", [E, EH], bf16, isOutput=False)
    wot = nc.declare_dram_parameter("wot", [EH, E], bf16, isOutput=False)
    maskt = nc.declare_dram_parameter("maskt", [128, NKC], f32, isOutput=False)
    bo_t = nc.declare_dram_parameter("bo_t", [1, E], f32, isOutput=False)
    out = nc.declare_dram_parameter("out", [QCC, E], f32, isOutput=True)
    # DRAM scratch for the per-sweep denominator-reciprocal broadcast
    dsc = [nc.dram_tensor(f"dsc{i}", [4, 512], bf16, kind="Internal") for i in range(2)]

    from contextlib import ExitStack

    with tile.TileContext(nc) as tc:
        with ExitStack() as _es:
            psum_s = _es.enter_context(tc.tile_pool(name="psum_s", bufs=2, space="PSUM"))
            psum_ctx = _es.enter_context(tc.tile_pool(name="psum_ctx", bufs=1, space="PSUM"))
            psum_d = _es.enter_context(tc.tile_pool(name="psum_d", bufs=1, space="PSUM"))
            psum_em = _es.enter_context(tc.tile_pool(name="psum_em", bufs=1, space="PSUM"))
            misc = _es.enter_context(tc.tile_pool(name="misc", bufs=1))
            pv = _es.enter_context(tc.tile_pool(name="pv", bufs=NKC))
            pctxn = _es.enter_context(tc.tile_pool(name="pctxn", bufs=NSW))
            pctxu = _es.enter_context(tc.tile_pool(name="pctxu", bufs=2))
            prs = _es.enter_context(tc.tile_pool(name="prs", bufs=2))
            pP = _es.enter_context(tc.tile_pool(name="pP", bufs=5))
            pwo = _es.enter_context(tc.tile_pool(name="pwo", bufs=NEH))
            post = _es.enter_context(tc.tile_pool(name="post", bufs=2))
            pdi = _es.enter_context(tc.tile_pool(name="pdi", bufs=2))
            pbt = _es.enter_context(tc.tile_pool(name="pbt", bufs=2))

            mask_tile = misc.tile([128, NKC], f32)
            nc.sync.dma_start(mask_tile[:], maskt[:])
            bo_tile = misc.tile([128, E], f32)
            bo_bcast = bass.AP(tensor=bo_t, offset=0, ap=[[0, 128], [1, E]])
            nc.sync.dma_start(bo_tile[:], bo_bcast)
            ones_tile = misc.tile([128, 32], bf16)
            nc.vector.memset(ones_tile[:], 1.0)
            # scratch for PE warm-up matmuls (HAM un-throttle before prelude)
            warm_sb = misc.tile([128, 512], bf16)
            nc.vector.memset(warm_sb[:], 0.0)
            # per-key DVE-exp bias: mask * SCH_A + SCH_B
            mask2_tile = misc.tile([128, NKC], f32)
            nc.vector.tensor_scalar(
                out=mask2_tile[:], in0=mask_tile[:],
                scalar1=SCH_A, scalar2=SCH_B, op0=mult, op1=add,
            )
            # warm the PE clock gate with dummy matmuls on zeroed SBUF while
            # input DMAs land; also pre-load the ACT exp table
            warm_ps = psum_em.tile([128, 512], f32, tag="em")
            for _ in range(20):
                nc.tensor.matmul(
                    warm_ps[:], warm_sb[:, 0:128], warm_sb[:],
                    start=True, stop=True,
                )
            warm_act = misc.tile([128, 1], bf16)
            nc.scalar.activation(warm_act[:], mask_tile[:, 0:1], Exp, scale=1.0)

            wo_tiles = [pwo.tile([128, E], bf16, name=f"wo{e}", tag="wo") for e in range(NEH)]

            v_tiles = [None] * NKC
            ctxn_tiles = [None] * NSW
            k_tiles = [None] * NPAIR
            q_tiles = [None] * NSW

            with ExitStack() as _es2:
                px = _es2.enter_context(tc.tile_pool(name="px", bufs=NEC))
                pwq = _es2.enter_context(tc.tile_pool(name="pwq", bufs=NEC))
                pwk = _es2.enter_context(tc.tile_pool(name="pwk", bufs=NEC))
                pwv = _es2.enter_context(tc.tile_pool(name="pwv", bufs=NEC))
                pk = _es2.enter_context(tc.tile_pool(name="pk", bufs=2))
                pq = _es2.enter_context(tc.tile_pool(name="pq", bufs=2))

                # DMA order is consumption order: the prelude K-proj needs
                # x cols 0:1024 and Wk's first 128-col block first — issue
                # those before the bulk so TensorE starts ~20us earlier.
                x_tiles = [px.tile([128, SEQ], bf16, name=f"x{e}", tag="x") for e in range(NEC)]
                wk_tiles = [pwk.tile([128, EH], bf16, name=f"wk{e}", tag="wk") for e in range(NEC)]
                wq_tiles = [pwq.tile([128, EH], bf16, name=f"wq{e}", tag="wq") for e in range(NEC)]
                wv_tiles = [pwv.tile([128, EH], bf16, name=f"wv{e}", tag="wv") for e in range(NEC)]

                for e in range(NEC):
                    nc.sync.dma_start(x_tiles[e][:, 0:1024], xt[128 * e:128 * e + 128, 0:1024])
                for e in range(NEC):
                    nc.sync.dma_start(wk_tiles[e][:, 0:128], wkt[128 * e:128 * e + 128, 0:128])
                for e in range(NEC):
                    nc.sync.dma_start(wq_tiles[e][:, 0:128], wqt[128 * e:128 * e + 128, 0:128])
                for e in range(NEC):
                    nc.sync.dma_start(wv_tiles[e][:], wvt[128 * e:128 * e + 128, :])
                for c in range(1, 4):
                    for e in range(NEC):
                        nc.sync.dma_start(
                            x_tiles[e][:, 1024 * c:1024 * c + 1024],
                            xt[128 * e:128 * e + 128, 1024 * c:1024 * c + 1024],
                        )
                for e in range(NEC):
                    nc.sync.dma_start(wk_tiles[e][:, 128:EH], wkt[128 * e:128 * e + 128, 128:EH])
                for e in range(NEC):
                    nc.sync.dma_start(wq_tiles[e][:, 128:EH], wqt[128 * e:128 * e + 128, 128:EH])
                for e in range(NEH):
                    nc.sync.dma_start(wo_tiles[e][:], wot[128 * e:128 * e + 128, :])

                def emit_k_sub(p, nt2):
                    # K^T cols [1024*nt2, +1024) for pair p: the two 512-col
                    # chunks accumulate through the em and d banks with a
                    # shared stationary per e-step, so the chains interleave
                    if k_tiles[p] is None:
                        k_tiles[p] = pk.tile([128, SEQ], bf16, name=f"k{p}", tag="k")
                    kt = k_tiles[p]
                    c0 = 1024 * nt2
                    ps1 = psum_em.tile([128, 512], f32, tag="em")
                    ps2 = psum_d.tile([128, 512], f32, tag="d")
                    for e in range(NEC):
                        for (ps, cc) in ((ps1, c0), (ps2, c0 + 512)):
                            nc.tensor.matmul(
                                ps[:],
                                wk_tiles[e][:, 128 * p:128 * p + 128],
                                x_tiles[e][:, cc:cc + 512],
                                start=(e == 0), stop=(e == NEC - 1),
                            )
                    nc.vector.tensor_copy(out=kt[:, c0:c0 + 512], in_=ps1[:])
                    nc.vector.tensor_copy(out=kt[:, c0 + 512:c0 + 1024], in_=ps2[:])

                def emit_q_sub(s):
                    # q for sweep s runs during sweep s-1; when the d bank is
                    # free there (s in 0/1/2/4) pair the two 512-chunks across
                    # em+d with a shared stationary per e-step, else borrow a
                    # score-pool buffer
                    p, qh2 = s // 2, s % 2
                    q_tiles[s] = pq.tile([128, QPC], bf16, name=f"q{s}", tag="q")
                    qt = q_tiles[s]
                    if s == 0:
                        # prelude: psum_s is free; keep em+d clear for the
                        # concurrent K chain
                        ps = psum_s.tile([128, 1024], f32, tag="ps")
                        for e in range(NEC):
                            for g in range(2):
                                nc.tensor.matmul(
                                    ps[:, 512 * g:512 * g + 512],
                                    wq_tiles[e][:, 128 * p:128 * p + 128],
                                    x_tiles[e][:, 512 * g:512 * g + 512],
                                    start=(e == 0), stop=(e == NEC - 1),
                                )
                        nc.vector.tensor_copy(out=qt[:], in_=ps[:])
                    elif s in (1, 2, 4):
                        ps1 = psum_em.tile([128, 512], f32, tag="em")
                        ps2 = psum_d.tile([128, 512], f32, tag="d")
                        for e in range(NEC):
                            for (ps, cc) in ((ps1, 0), (ps2, 512)):
                                nc.tensor.matmul(
                                    ps[:],
                                    wq_tiles[e][:, 128 * p:128 * p + 128],
                                    x_tiles[e][:, 1024 * qh2 + cc:1024 * qh2 + cc + 512],
                                    start=(e == 0), stop=(e == NEC - 1),
                                )
                        nc.vector.tensor_copy(out=qt[:, 0:512], in_=ps1[:])
                        nc.vector.tensor_copy(out=qt[:, 512:1024], in_=ps2[:])
                    else:
                        ps1 = psum_em.tile([128, 512], f32, tag="em")
                        ps = psum_s.tile([128, 1024], f32, tag="ps")
                        for e in range(NEC):
                            for (pp, cc) in ((ps1, 0), (ps, 512)):
                                nc.tensor.matmul(
                                    pp[:, 0:512] if pp is ps1 else pp[:, 0:512],
                                    wq_tiles[e][:, 128 * p:128 * p + 128],
                                    x_tiles[e][:, 1024 * qh2 + cc:1024 * qh2 + cc + 512],
                                    start=(e == 0), stop=(e == NEC - 1),
                                )
                        nc.vector.tensor_copy(out=qt[:, 0:512], in_=ps1[:])
                        nc.vector.tensor_copy(out=qt[:, 512:1024], in_=ps[:, 0:512])

                def emit_v(j):
                    # V rows [128*j, +128): [128 k, 384 d]
                    v_tiles[j] = pv.tile([128, EH], bf16, name=f"v{j}", tag="v")
                    ps = psum_em.tile([128, EH], f32, tag="em")
                    for e in range(NEC):
                        nc.tensor.matmul(
                            ps[:],
                            x_tiles[e][:, 128 * j:128 * j + 128],
                            wv_tiles[e][:],
                            start=(e == 0), stop=(e == NEC - 1),
                        )
                    nc.vector.tensor_copy(out=v_tiles[j][:], in_=ps[:])

                def emit_v2(j1, j2):
                    # two V chunks, accumulation chains interleaved across the
                    # emission bank and the (idle-in-sweep-0) denominator bank
                    # so each chain's LDW/RAW latency hides behind the other
                    v_tiles[j1] = pv.tile([128, EH], bf16, name=f"v{j1}", tag="v")
                    v_tiles[j2] = pv.tile([128, EH], bf16, name=f"v{j2}", tag="v")
                    ps1 = psum_em.tile([128, EH], f32, tag="em")
                    ps2 = psum_d.tile([128, EH], f32, tag="d")
                    for e in range(NEC):
                        for (jj, ps) in ((j1, ps1), (j2, ps2)):
                            nc.tensor.matmul(
                                ps[:],
                                x_tiles[e][:, 128 * jj:128 * jj + 128],
                                wv_tiles[e][:],
                                start=(e == 0), stop=(e == NEC - 1),
                            )
                    nc.vector.tensor_copy(out=v_tiles[j1][:], in_=ps1[:])
                    nc.vector.tensor_copy(out=v_tiles[j2][:], in_=ps2[:])


                K_AT = {4: 0, 10: 1, 16: 2, 22: 3}
                K_AT0 = {3: 1, 11: 2, 19: 3}  # pair-0 tail emitted inside sweep 0 (odd j: V pairs own even j)

                def emit_out(t8):
                    # output-projection chunk for queries [128*t8,+128),
                    # interleaved into sweep 5. Runs through the em+d banks
                    # (free in sweep 5) with a shared stationary per c-step so
                    # the two column-group chains pipeline, and leaves the
                    # score-pool rotation untouched.
                    qh2o, tc8 = t8 // 8, t8 % 8
                    ps1 = psum_em.tile([128, 512], f32, tag="em")
                    ps2 = psum_d.tile([128, 256], f32, tag="d")
                    for c in range(NPAIR):
                        cx = ctxn_tiles[2 * c + qh2o][:, 128 * tc8:128 * tc8 + 128]
                        nc.tensor.matmul(
                            ps1[:], cx, wo_tiles[c][:, 0:512],
                            start=(c == 0), stop=(c == NPAIR - 1),
                        )
                        nc.tensor.matmul(
                            ps2[:], cx, wo_tiles[c][:, 512:768],
                            start=(c == 0), stop=(c == NPAIR - 1),
                        )
                    st = post.tile([128, E], f32)
                    nc.vector.tensor_tensor(
                        out=st[:, 0:512], in0=ps1[:], in1=bo_tile[:, 0:512], op=add
                    )
                    nc.vector.tensor_tensor(
                        out=st[:, 512:768], in0=ps2[:], in1=bo_tile[:, 512:768], op=add
                    )
                    nc.sync.dma_start(out[128 * t8:128 * t8 + 128, :], st[:])

                def sweep(sw, first):
                    p, qh2 = sw // 2, sw % 2
                    kt, qt = k_tiles[p], q_tiles[sw]
                    ctx_ps = psum_ctx.tile([128, 1024], f32)
                    # sweep 0 has no PE-denominator tiles; its d bank doubles
                    # as a second V-emission buffer during the j-loop
                    d_ps = (psum_d.tile([128, 512], f32, name="d_ps", tag="d")
                            if sw in (2, 4) else None)
                    a, b = 2 * p, 2 * p + 1

                    # static denominator bookkeeping per h: which j's hit PE
                    pe_js = [[j for j in range(NKC) if DEN_PE(sw, j, h)] for h in range(2)]
                    dve_js = [[j for j in range(NKC) if not DEN_PE(sw, j, h)] for h in range(2)]
                    rs_tiles = [None, None]

                    def emit_scores(j):
                        # scores + exp for both h-halves of chunk j. Both exp
                        # ops issue BEFORE the running-sum adds so the DVE's
                        # h1 exp runs concurrently with ACT's h0 exp instead
                        # of queuing behind an add that waits on ACT's output
                        res = []
                        for h in range(2):
                            s = psum_s.tile([128, 1024], f32, tag="ps")
                            nc.tensor.matmul(
                                s[:, 0:512],
                                kt[0:64, 128 * j:128 * j + 128],
                                qt[0:64, 512 * h:512 * h + 512],
                                start=True, stop=True, tile_position=(0, 0),
                            )
                            nc.tensor.matmul(
                                s[:, 512:1024],
                                kt[64:128, 128 * j:128 * j + 128],
                                qt[64:128, 512 * h:512 * h + 512],
                                start=True, stop=True, tile_position=(64, 0),
                            )
                            pt = pP.tile([128, 1024], bf16)
                            if EXP_DVE(sw, j, h):
                                nc.vector.tensor_scalar(
                                    out=pt[:].bitcast(i16), in0=s[:],
                                    scalar1=SCH_A * 0.125,
                                    scalar2=mask2_tile[:, j:j + 1],
                                    op0=mult, op1=add,
                                )
                            else:
                                nc.scalar.activation(
                                    pt[:], s[:], Exp,
                                    bias=mask_tile[:, j:j + 1], scale=0.125,
                                )
                            res.append(pt)
                        for h in range(2):
                            if not DEN_PE(sw, j, h):
                                if j == dve_js[h][0]:
                                    rs_tiles[h] = prs.tile([128, 1024], bf16, name=f"rs{h}", tag="rs")
                                    nc.vector.tensor_copy(
                                        out=rs_tiles[h][:], in_=res[h][:]
                                    )
                                else:
                                    nc.vector.tensor_tensor(
                                        out=rs_tiles[h][:], in0=rs_tiles[h][:],
                                        in1=res[h][:], op=add,
                                    )
                        return res

                    # software pipeline: scores(j+1) issue before PV(j) so the
                    # PE never head-blocks on exp(j) completing
                    pts = emit_scores(0)
                    for j in range(NKC):
                        if sw == NSW - 1 and j >= 2 and (j - 2) % 3 == 0:
                            t8e = (j - 2) // 3
                            if t8e < 8:
                                emit_out(t8e)
                        # in the PE-bound first sweep, issue the V/K chains
                        # (and their DVE casts) ahead of the exp/add block so
                        # the em+d bank rotation is never gated by adds that
                        # wait on ACT output
                        if first and j % 2 == 0 and j < NKC - 2:
                            emit_v2(j + 1, j + 2)
                        elif first and j == NKC - 2:
                            emit_v(NKC - 1)
                        if first and j in K_AT0:
                            emit_k_sub(0, K_AT0[j])
                        nxt = emit_scores(j + 1) if j < NKC - 1 else None
                        # both PVs after both scores: halves PE tiling-mode
                        # switches (row->col once per j instead of twice)
                        for h in range(2):
                            pt = pts[h]
                            nc.tensor.matmul(
                                ctx_ps[0:64, 512 * h:512 * h + 512],
                                v_tiles[j][:, 64 * a:64 * a + 64],
                                pt[:, 0:512],
                                start=(j == 0), stop=(j == NKC - 1),
                                skip_group_check=True,
                                tile_position=(0, 0),
                            )
                            nc.tensor.matmul(
                                ctx_ps[64:128, 512 * h:512 * h + 512],
                                v_tiles[j][:, 64 * b:64 * b + 64],
                                pt[:, 512:1024],
                                start=(j == 0), stop=(j == NKC - 1),
                                skip_group_check=True,
                                tile_position=(0, 64),
                            )
                            if DEN_PE(sw, j, h):
                                st_flag = (j == pe_js[h][0])
                                sp_flag = (j == pe_js[h][-1]) and not dve_js[h]
                                nc.tensor.matmul(
                                    d_ps[32 * h:32 * h + 32, :],
                                    ones_tile[:], pt[:, 0:512],
                                    start=st_flag, stop=sp_flag,
                                    skip_group_check=True,
                                    tile_position=(0, 32 * h),
                                )
                                nc.tensor.matmul(
                                    d_ps[64 + 32 * h:96 + 32 * h, :],
                                    ones_tile[:], pt[:, 512:1024],
                                    start=st_flag, stop=sp_flag,
                                    skip_group_check=True,
                                    tile_position=(0, 64 + 32 * h),
                                )
                        pts = nxt
                        if j == 27 and sw < NSW - 1:
                            emit_q_sub(sw + 1)
                        if qh2 == 1 and p < NPAIR - 1 and j in K_AT:
                            emit_k_sub(p + 1, K_AT[j])
                    # fold the DVE running sums into the PSUM denominator
                    if d_ps is None:
                        d_ps = psum_d.tile([128, 512], f32, name="d_ps0", tag="d")
                    for h in range(2):
                        if dve_js[h]:
                            st_flag = not pe_js[h]
                            nc.tensor.matmul(
                                d_ps[32 * h:32 * h + 32, :],
                                ones_tile[:], rs_tiles[h][:, 0:512],
                                start=st_flag, stop=True,
                                skip_group_check=True,
                                tile_position=(0, 32 * h),
                            )
                            nc.tensor.matmul(
                                d_ps[64 + 32 * h:96 + 32 * h, :],
                                ones_tile[:], rs_tiles[h][:, 512:1024],
                                start=st_flag, stop=True,
                                skip_group_check=True,
                                tile_position=(0, 64 + 32 * h),
                            )
                    # evacuate ctx unnormalized right away to free PSUM for the
                    # next sweep; reciprocal + normalize run off the critical
                    # path, overlapped with the next sweep's compute
                    ctxu = pctxu.tile([128, 1024], bf16, tag="ctxu")
                    nc.vector.tensor_copy(out=ctxu[:], in_=ctx_ps[:])
                    dinv = pdi.tile([128, 1024], bf16, tag="di")
                    with nc.allow_low_precision(reason="softmax denominators are O(1e3); bf16 reciprocal adds <0.4% rel err"):
                        nc.vector.reciprocal(dinv[:, 0:512], d_ps[:])
                    # broadcast the packed reciprocals into bt[128,1024] via a
                    # DRAM bounce (SBUF sources cannot have stride-0 partitions,
                    # DRAM sources can); latency hides under the next sweep
                    ds = dsc[sw % 2]
                    bt = pbt.tile([128, 1024], bf16, tag="bt")
                    nc.sync.dma_start(ds[:], dinv[0:128:32, 0:512])
                    for (row4, r0, c0) in ((0, 0, 0), (1, 0, 512),
                                           (2, 64, 0), (3, 64, 512)):
                        src_b = bass.AP(tensor=ds, offset=row4 * 512,
                                        ap=[[0, 64], [1, 512]])
                        nc.sync.dma_start(bt[r0:r0 + 64, c0:c0 + 512], src_b)
                    ctxn_tiles[sw] = pctxn.tile([128, 1024], bf16, name=f"ctxn{sw}", tag="ctxn")
                    nc.vector.tensor_tensor(
                        out=ctxn_tiles[sw][:], in0=ctxu[:], in1=bt[:], op=mult
                    )

                # prelude: minimal K/Q/V for sweep 0 to start; the rest of
                # pair-0 K interleaves into sweep 0 (K_AT0)
                emit_k_sub(0, 0)
                emit_q_sub(0)
                v_tiles[0] = pv.tile([128, EH], bf16, name="v0", tag="v")
                v0ps = psum_s.tile([128, EH], f32, name="v0ps", tag="ps")
                for e in range(NEC):
                    nc.tensor.matmul(
                        v0ps[:],
                        x_tiles[e][:, 0:128],
                        wv_tiles[e][:],
                        start=(e == 0), stop=(e == NEC - 1),
                    )
                nc.vector.tensor_copy(out=v_tiles[0][:], in_=v0ps[:])

                for sw in range(NSW):
                    sweep(sw, first=(sw == 0))

            # phase 3: remaining output-projection chunks (qh2=1; the qh2=0
            # half was interleaved into sweep 5)
            for wave in ((8, 9, 10), (11, 12, 13), (14, 15)):
                # up to three concurrent accumulation chains: two through the
                # score pool, one split across the em+d banks
                regions = {}
                for i, t8 in enumerate(wave):
                    if i < 2:
                        ps = psum_s.tile([128, 1024], f32, name=f"o{t8}", tag="ps")
                        regions[t8] = ((ps[:, 0:512], 0, 512), (ps[:, 512:768], 512, 256))
                    else:
                        ps1 = psum_em.tile([128, 512], f32, tag="em")
                        ps2 = psum_d.tile([128, 256], f32, tag="d")
                        regions[t8] = ((ps1[:], 0, 512), (ps2[:], 512, 256))
                for c in range(NPAIR):
                    for t8 in wave:
                        qh2, tc8 = t8 // 8, t8 % 8
                        cx = ctxn_tiles[2 * c + qh2][:, 128 * tc8:128 * tc8 + 128]
                        for (reg, c0, w) in regions[t8]:
                            nc.tensor.matmul(
                                reg, cx, wo_tiles[c][:, c0:c0 + w],
                                start=(c == 0), stop=(c == NPAIR - 1),
                            )
                for t8 in wave:
                    st = post.tile([128, E], f32)
                    for (reg, c0, w) in regions[t8]:
                        nc.vector.tensor_tensor(
                            out=st[:, c0:c0 + w], in0=reg, in1=bo_tile[:, c0:c0 + w], op=add
                        )
                    nc.sync.dma_start(out[128 * t8:128 * t8 + 128, :], st[:])

    _legalize_waits(nc, mybir)
    return nc


def _legalize_waits(nc, mybir, mm_limit=1, other_limit=1, nop_limit=1):
    # walrus rejects instructions with more sync-wait commands than the ISA
    # struct has slots (Matmult: 1). Hoist extra waits onto preceding NoOps
    # on the same engine (engines are in-order, so this is equivalent).
    for bbname, bbw in nc.bb_map.items():
        bb = bbw.bb
        insts = list(bb.instructions)
        out = []
        changed = False
        for inst in insts:
            si = inst.sync_info
            waits = list(si.on_wait) if si is not None else []
            limit = (
                mm_limit
                if isinstance(inst, (mybir.InstMatmult, mybir.InstLdweights))
                else other_limit
            )
            if len(waits) > limit:
                changed = True
                extra = waits[limit:]
                while extra:
                    chunk, extra = extra[:nop_limit], extra[nop_limit:]
                    nop = mybir.InstNoOp(
                        name=nc.get_next_instruction_name(),
                        ins=[],
                        outs=[],
                        sync_info=mybir.SyncInfo(on_wait=chunk, on_update=[]),
                        engine=inst.engine,
                        bass_nofuse=True,
                    )
                    nc.inst_map[nop.name] = nop
                    out.append(nop)
                si.on_wait = waits[:limit]
            out.append(inst)
        if changed:
            bb.instructions = out
    return nc


def kernel(**inputs):
    from concourse.bass_utils import run_bass_kernel_spmd

    hs = np.asarray(inputs["hidden_states"], dtype=np.float32)
    am = np.asarray(inputs["attention_mask"], dtype=np.float32)
    Wq = np.asarray(inputs["Wq"], dtype=np.float32)
    Wk = np.asarray(inputs["Wk"], dtype=np.float32)
    Wv = np.asarray(inputs["Wv"], dtype=np.float32)
    Wo = np.asarray(inputs["Wo"], dtype=np.float32)
    bo = np.asarray(inputs["bo"], dtype=np.float32)

    if "nc" not in _cache:
        _cache["nc"] = _build()
    nc = _cache["nc"]

    bo2d = np.ascontiguousarray(bo.reshape(1, E))
    zeros2d = np.zeros((1, E), dtype=np.float32)

    # per-head-half weight slices: core (b, qh, hh) computes heads
    # [6*hh, 6*hh+6) for queries [2048*qh, +2048) of batch b
    WqTh = [np.ascontiguousarray(Wq[EH * hh:EH * hh + EH, :].T).astype(BF16) for hh in range(2)]
    WkTh = [np.ascontiguousarray(Wk[EH * hh:EH * hh + EH, :].T).astype(BF16) for hh in range(2)]
    WvTh = [np.ascontiguousarray(Wv[EH * hh:EH * hh + EH, :].T).astype(BF16) for hh in range(2)]
    WoTh = [np.ascontiguousarray(Wo[:, EH * hh:EH * hh + EH].T).astype(BF16) for hh in range(2)]

    in_maps = []
    xtr_c = {}
    for c in range(8):
        b, qh, hh = c // 4, (c // 2) % 2, c % 2
        qs = QCC * qh
        if (b, qh) not in xtr_c:
            xr = np.roll(hs[b].T, -qs, axis=1).astype(BF16)
            mr = np.roll(am[b, 0, 0], -qs)
            xtr_c[(b, qh)] = (
                np.ascontiguousarray(xr),
                np.ascontiguousarray(mr.reshape(NKC, 128).T),
            )
        xtr, mtile = xtr_c[(b, qh)]
        in_maps.append({
            "xt": xtr,
            "wqt": WqTh[hh], "wkt": WkTh[hh], "wvt": WvTh[hh],
            "wot": WoTh[hh], "maskt": mtile,
            "bo_t": bo2d if hh == 0 else zeros2d,
        })

    res = run_bass_kernel_spmd(nc, in_maps, list(range(8)))
    _cache["last_res"] = res
    full = np.empty((BSZ, SEQ, E), dtype=np.float32)
    for b in range(BSZ):
        for qh in range(2):
            c0 = b * 4 + qh * 2
            part = res.results[c0]["out"] + res.results[c0 + 1]["out"]
            full[b, QCC * qh:QCC * qh + QCC, :] = part
    return full
